# revision 3
# baseline (speedup 1.0000x reference)
"""TRN2 Bass kernel v2: masked MHA block (B=4, S=2048, C=768, H=12).

Sharding: 8 cores = 4 batches x 2 head-groups (6 heads each), host combines.

Per-core design (all SBUF data fp16, PSUM fp32):
  phase 1:  kT, v projections + qT(qb=0)  [PE + ACT/DVE psum-evac copies]
  loop over 4 q-blocks of 512:
    qT(qb+1) projection
    3 head-pairs x 8 kc-pairs:
      scores:  sc[128, 2, 512] psum  (2 matmuls per head, K=64, heads packed
               on PE rows via partition offset 0/64)
      exp:     ACT psum->sbuf fp16 (one inst per head, [128, 1024])
      mask:    DVE fp16 mul by binary keep-mask (2x mode), shared per batch
      AV:      av[65, 512] psum accumulated over 16 kc (ones col = denom)
    normalize: gpsimd gather denom rows -> DVE recip -> Pool broadcast ->
               DVE mul psum->attn_sb fp16
    y proj for the 4 s-chunks of this q-block
Host: fp16 prep of inputs, sums per-batch partials, adds b_proj.
"""

from contextlib import ExitStack

import ml_dtypes
import numpy as np

import concourse.tile as tile
from concourse import bacc, mybir
from concourse.bass_utils import run_bass_kernel_spmd

F32 = mybir.dt.float32
F16 = mybir.dt.bfloat16  # bf16: same PE rate as fp16, less power/throttle

B, S, C, H = 4, 2048, 768, 12
HD = 64
H_PER_CORE = 6
D_CORE = H_PER_CORE * HD  # 384
QBLK = 512
N_CORES = 8
KC = S // 128           # 16 key chunks
KCP = KC // 2           # 8 key-chunk pairs
QB = S // QBLK          # 4 q blocks
NB = S // QBLK
ST = S // 128           # 16 s chunks


def _build_kernel():
    nc = bacc.Bacc(
        trn_type="TRN2", target_bir_lowering=False, debug=False, num_devices=N_CORES
    )

    xT = nc.dram_tensor("xT", [C, S], F16, kind="ExternalInput").ap()
    wq = nc.dram_tensor("wq", [C, D_CORE], F16, kind="ExternalInput").ap()
    wk = nc.dram_tensor("wk", [C, D_CORE], F16, kind="ExternalInput").ap()
    wv = nc.dram_tensor("wv", [C, D_CORE], F16, kind="ExternalInput").ap()
    wproj = nc.dram_tensor("wproj", [D_CORE, C], F16, kind="ExternalInput").ap()
    vones = nc.dram_tensor("vones", [128, ST * H_PER_CORE], F16, kind="ExternalInput").ap()
    maskT = nc.dram_tensor("maskT", [S, S], F16, kind="ExternalInput").ap()
    y = nc.dram_tensor("y", [S, C], F32, kind="ExternalOutput").ap()

    maskT_r = maskT.rearrange("(kc p) q -> p kc q", p=128)
    y_r = y.rearrange("(st p) o -> st p o", p=128)

    with tile.TileContext(nc) as tc, ExitStack() as ctx:
        consts = ctx.enter_context(tc.tile_pool(name="consts", bufs=1))
        big = ctx.enter_context(tc.tile_pool(name="big", bufs=1))
        mpool = ctx.enter_context(tc.tile_pool(name="mask", bufs=16))
        ppool = ctx.enter_context(tc.tile_pool(name="pT", bufs=3))
        pmpool = ctx.enter_context(tc.tile_pool(name="pTm", bufs=3))
        npool = ctx.enter_context(tc.tile_pool(name="norm", bufs=2))
        ypool = ctx.enter_context(tc.tile_pool(name="ysb", bufs=2))
        ps_sc = ctx.enter_context(tc.tile_pool(name="ps_sc", bufs=2, space="PSUM"))
        ps_av = ctx.enter_context(tc.tile_pool(name="ps_av", bufs=2, space="PSUM"))
        ps_qy = ctx.enter_context(tc.tile_pool(name="ps_qy", bufs=2, space="PSUM"))

        # ---- weights + x in SBUF ----
        # wk + xT first (kT proj is the critical path); x split per k-chunk
        # across two queues so the first matmul can start early.
        wq_sb = consts.tile([128, 6, D_CORE], F16)
        wk_sb = consts.tile([128, 6, D_CORE], F16)
        wv_sb = consts.tile([128, 6, D_CORE], F16)
        wproj_sb = consts.tile([128, 3, C], F16)
        xT_sb = big.tile([128, 6, S], F16)
        nc.sync.dma_start(wk_sb[:], wk.rearrange("(t p) d -> p t d", p=128))
        xT_r = xT.rearrange("(t p) s -> p t s", p=128)
        for k in range(6):
            eng = nc.sync if k % 2 == 0 else nc.scalar
            eng.dma_start(xT_sb[:, k, :], xT_r[:, k, :])
        nc.gpsimd.dma_start(wv_sb[:], wv.rearrange("(t p) d -> p t d", p=128))
        nc.gpsimd.dma_start(wq_sb[:], wq.rearrange("(t p) d -> p t d", p=128))
        nc.gpsimd.dma_start(wproj_sb[:], wproj.rearrange("(t p) o -> p t o", p=128))

        qT_sb = big.tile([128, 3, S], F16)
        kT_sb = big.tile([128, 3, S], F16)
        vaug = big.tile([128, KC, H_PER_CORE, HD + 1], F16)
        attn_sb = big.tile([128, 3, S], F16)

        nc.gpsimd.dma_start(
            vaug[:, :, :, HD], vones.rearrange("p (st h) -> p st h", h=H_PER_CORE)
        )

        # ---- phase 1: kT, v, qT(0) ----
        def proj_qk(w_sb, dst, nb):
            for m in range(3):
                ps = ps_qy.tile([128, QBLK], F32, tag="qy", name="qk")
                for k in range(6):
                    nc.tensor.matmul(
                        ps[:],
                        w_sb[:, k, m * 128:(m + 1) * 128],
                        xT_sb[:, k, nb * QBLK:(nb + 1) * QBLK],
                        start=(k == 0),
                        stop=(k == 5),
                    )
                nc.scalar.copy(dst[:, m, nb * QBLK:(nb + 1) * QBLK], ps[:])

        for nb in range(NB):
            proj_qk(wk_sb, kT_sb, nb)
        for st in range(ST):
            psv = ps_sc.tile([128, D_CORE], F32, tag="sc", name="psv")
            for k in range(6):
                nc.tensor.matmul(
                    psv[:],
                    xT_sb[:, k, st * 128:(st + 1) * 128],
                    wv_sb[:, k, :],
                    start=(k == 0),
                    stop=(k == 5),
                )
            nc.vector.tensor_copy(
                vaug[:, st, :, 0:HD],
                psv[:].rearrange("p (h d) -> p h d", h=H_PER_CORE),
            )
        proj_qk(wq_sb, qT_sb, 0)

        # ---- main loop over q blocks ----
        def load_mask(qb_i):
            tiles = []
            for kcp in range(KCP):
                mt = mpool.tile([128, 2, QBLK], F16, tag="mask", name="mt")
                nc.gpsimd.dma_start(
                    mt[:],
                    maskT_r[:, 2 * kcp:2 * kcp + 2,
                            qb_i * QBLK:(qb_i + 1) * QBLK],
                )
                tiles.append(mt)
            return tiles

        mask_tiles = load_mask(0)
        for qb in range(QB):
            q0 = qb * QBLK
            next_mask = load_mask(qb + 1) if qb + 1 < QB else None
            if qb + 1 < QB:
                proj_qk(wq_sb, qT_sb, qb + 1)

            for pair in range(3):
                hA, hB = 2 * pair, 2 * pair + 1
                av = [
                    ps_av.tile([HD + 1, QBLK], F32, tag="av", name=f"av{i}")
                    for i in range(2)
                ]
                # score/exp/mask blocks per head (long same-config PE runs),
                # then all AV matmuls as one block (one PE config switch).
                pTms = []
                for i_h, h in ((0, hA), (1, hB)):
                    r0 = (h % 2) * HD
                    for kcp in range(KCP):
                        sc = ps_sc.tile([128, 2, QBLK], F32, tag="sc", name="sc")
                        for i in range(2):
                            kc = 2 * kcp + i
                            nc.tensor.matmul(
                                sc[:, i, :],
                                kT_sb[r0:r0 + HD, pair, kc * 128:(kc + 1) * 128],
                                qT_sb[r0:r0 + HD, pair, q0:q0 + QBLK],
                                start=True,
                                stop=True,
                                tile_position=(r0, 0),
                            )
                        pT = ppool.tile([128, 2, QBLK], F16, tag="pT", name="pT")
                        nc.scalar.activation(
                            pT[:], sc[:], mybir.ActivationFunctionType.Exp
                        )
                        pTm = pmpool.tile(
                            [128, 2, QBLK], F16, tag="pTm", name="pTm", bufs=17
                        )
                        nc.vector.tensor_mul(pTm[:], pT[:], mask_tiles[kcp][:])
                        pTms.append((i_h, h, kcp, pTm))
                for i_h, h, kcp, pTm in pTms:
                    for i in range(2):
                        kc = 2 * kcp + i
                        nc.tensor.matmul(
                            av[i_h][:],
                            vaug[:, kc, h, :],
                            pTm[:, i, :],
                            start=(kcp == 0 and i == 0),
                            stop=(kcp == KCP - 1 and i == 1),
                        )

                # normalization for this head pair
                av_sb = npool.tile([HD + 1, 2, QBLK], F32, tag="av_sb", name="av_sb")
                for i_h in range(2):
                    nc.vector.tensor_copy(av_sb[:, i_h, :], av[i_h][:])
                dstack = npool.tile([1, 2, QBLK], F32, tag="dstack", name="dstack")
                for i_h in range(2):
                    nc.gpsimd.dma_start(
                        dstack[:, i_h, :], av_sb[HD:HD + 1, i_h, :]
                    )
                recip = npool.tile([1, 2, QBLK], F32, tag="recip", name="recip")
                nc.vector.reciprocal_approx_fast(recip[:], dstack[:])
                for i_h, h in ((0, hA), (1, hB)):
                    r0 = (h % 2) * HD
                    bc = npool.tile([HD, QBLK], F32, tag="bc", name="bc")
                    nc.gpsimd.partition_broadcast(bc[:], recip[:, i_h, :])
                    nc.vector.tensor_mul(
                        attn_sb[r0:r0 + HD, pair, q0:q0 + QBLK],
                        av_sb[0:HD, i_h, :],
                        bc[:],
                    )

            # y projection for this q block
            for st4 in range(4):
                st = qb * 4 + st4
                y_sb = ypool.tile([128, C], F32, tag="ysb", name="y_sb")
                for half in range(2):
                    ps = ps_qy.tile([128, QBLK], F32, tag="qy", name="psy")
                    for k3 in range(3):
                        nc.tensor.matmul(
                            ps[:, 0:D_CORE],
                            attn_sb[:, k3, st * 128:(st + 1) * 128],
                            wproj_sb[:, k3, half * D_CORE:(half + 1) * D_CORE],
                            start=(k3 == 0),
                            stop=(k3 == 2),
                        )
                    nc.vector.tensor_copy(
                        y_sb[:, half * D_CORE:(half + 1) * D_CORE],
                        ps[:, 0:D_CORE],
                    )
                    nc.sync.dma_start(
                        y_r[st][:, half * D_CORE:(half + 1) * D_CORE],
                        y_sb[:, half * D_CORE:(half + 1) * D_CORE],
                    )

            mask_tiles = next_mask

    nc.compile()
    return nc


def _prep_core_inputs(x, mask, w_qkv, w_proj, core):
    b, g = core // 2, core % 2
    scale = HD ** -0.5
    s0, s1 = D_CORE * g, D_CORE * (g + 1)
    return {
        "xT": np.ascontiguousarray(x[b].T).astype(ml_dtypes.bfloat16),
        "wq": np.ascontiguousarray((w_qkv[s0:s1, :] * scale).T).astype(ml_dtypes.bfloat16),
        "wk": np.ascontiguousarray(w_qkv[C + s0:C + s1, :].T).astype(ml_dtypes.bfloat16),
        "wv": np.ascontiguousarray(w_qkv[2 * C + s0:2 * C + s1, :].T).astype(ml_dtypes.bfloat16),
        "wproj": np.ascontiguousarray(w_proj[:, s0:s1].T).astype(ml_dtypes.bfloat16),
        "maskT": np.ascontiguousarray((1 - mask[b].T)).astype(ml_dtypes.bfloat16),
        "vones": np.ones((128, ST * H_PER_CORE), dtype=ml_dtypes.bfloat16),
    }


_NC_CACHE = {}


def get_nc():
    if "nc" not in _NC_CACHE:
        _NC_CACHE["nc"] = _build_kernel()
    return _NC_CACHE["nc"]


def _build_runner(nc):
    """Reusable jitted shard_map callable over the 8 cores."""
    import jax
    from jax.experimental.shard_map import shard_map
    from jax.sharding import Mesh, PartitionSpec

    from concourse.bass2jax import (
        _bass_exec_p,
        install_neuronx_cc_hook,
        partition_id_tensor,
    )

    install_neuronx_cc_hook()
    partition_name = nc.partition_id_tensor.name if nc.partition_id_tensor else None
    in_names, out_names, out_avals, zero_outs = [], [], [], []
    for alloc in nc.m.functions[0].allocations:
        if not isinstance(alloc, mybir.MemoryLocationSet):
            continue
        name = alloc.memorylocations[0].name
        if alloc.kind == "ExternalInput":
            if name != partition_name:
                in_names.append(name)
        elif alloc.kind == "ExternalOutput":
            out_names.append(name)
            shape = tuple(alloc.tensor_shape)
            dtype = mybir.dt.np(alloc.dtype)
            out_avals.append(jax.core.ShapedArray(shape, dtype))
            zero_outs.append(np.zeros(shape, dtype))
    n_params = len(in_names)
    all_in_names = list(in_names) + list(out_names)
    if partition_name is not None:
        all_in_names.append(partition_name)

    def _body(*args):
        operands = list(args)
        if partition_name is not None:
            operands.append(partition_id_tensor())
        outs = _bass_exec_p.bind(
            *operands,
            out_avals=tuple(out_avals),
            in_names=tuple(all_in_names),
            out_names=tuple(out_names),
            lowering_input_output_aliases=(),
            sim_require_finite=True,
            sim_require_nnan=True,
            nc=nc,
        )
        return tuple(outs)

    n_cores = nc.num_devices
    devices = jax.devices()[:n_cores]
    mesh = Mesh(np.asarray(devices), ("core",))
    in_specs = (PartitionSpec("core"),) * (n_params + len(out_names))
    out_specs = (PartitionSpec("core"),) * len(out_names)
    fn = jax.jit(
        shard_map(
            _body, mesh=mesh, in_specs=in_specs, out_specs=out_specs, check_rep=False
        ),
        keep_unused=True,
    )
    return fn, in_names, out_names, zero_outs


_RUNNER_CACHE = {}


def get_runner(nc, in_maps):
    """Return (fn, dev_args) for repeated dispatch of `nc` with `in_maps`."""
    import jax
    from jax.sharding import Mesh, NamedSharding, PartitionSpec

    key = id(nc)
    if key not in _RUNNER_CACHE:
        _RUNNER_CACHE[key] = _build_runner(nc)
    fn, in_names, out_names, zero_outs = _RUNNER_CACHE[key]
    n_cores = nc.num_devices
    mesh = Mesh(np.asarray(jax.devices()[:n_cores]), ("core",))
    shard = NamedSharding(mesh, PartitionSpec("core"))
    concat_in = [
        np.concatenate([np.asarray(in_maps[c][n]) for c in range(n_cores)], axis=0)
        for n in in_names
    ]
    dev_in = [jax.device_put(a, shard) for a in concat_in]
    zkey = ("zeros", key)
    if zkey not in _RUNNER_CACHE:
        concat_zeros = [
            np.zeros((n_cores * z.shape[0], *z.shape[1:]), z.dtype) for z in zero_outs
        ]
        _RUNNER_CACHE[zkey] = [jax.device_put(a, shard) for a in concat_zeros]
    return fn, dev_in + _RUNNER_CACHE[zkey]


def run_cached(nc, in_maps):
    """Execute via the cached runner; returns per-core result dicts."""
    fn, dev_args = get_runner(nc, in_maps)
    out_arrs = fn(*dev_args)
    _, _, out_names, zero_outs = _RUNNER_CACHE[id(nc)]
    n_cores = nc.num_devices
    fetched = [
        np.asarray(a).reshape(n_cores, *zero_outs[i].shape)
        for i, a in enumerate(out_arrs)
    ]
    return [
        {name: fetched[i][c] for i, name in enumerate(out_names)}
        for c in range(n_cores)
    ]


def make_in_maps(x, mask, w_qkv, w_proj):
    return [_prep_core_inputs(x, mask, w_qkv, w_proj, c) for c in range(N_CORES)]


def combine(results, b_proj):
    outs = []
    for b in range(B):
        outs.append(results[2 * b]["y"] + results[2 * b + 1]["y"] + b_proj[None, :])
    return np.stack(outs).astype(np.float32)


def kernel(x, mask, w_qkv, w_proj, b_proj):
    x = np.asarray(x, dtype=np.float32)
    mask = np.asarray(mask)
    w_qkv = np.asarray(w_qkv, dtype=np.float32)
    w_proj = np.asarray(w_proj, dtype=np.float32)
    b_proj = np.asarray(b_proj, dtype=np.float32)

    nc = get_nc()
    in_maps = make_in_maps(x, mask, w_qkv, w_proj)
    try:
        results = run_cached(nc, in_maps)
    except Exception:
        results = run_bass_kernel_spmd(nc, in_maps, list(range(N_CORES))).results
    return combine(results, b_proj)
